# revision 1
# baseline (speedup 1.0000x reference)
"""GQA attention layer (B=2, L=2048, D=4096, H=32, KH=8, HD=128) on 8 TRN2 cores.

Sharding: tensor-parallel over KV heads (1 kv head + 4 q heads per core).
Per core: QKV projection (fp32r matmuls, x^T stationary), per-head RMSNorm +
RoPE folded into host-precomputed tables, flash-style attention in transposed
(S^T) layout with causal block skipping, AllToAll to redistribute attention
outputs token-wise, then the output projection for this core's 512-token slice.
Host assembles the 8 token slices.
"""
import numpy as np

import concourse.bass as bass
import concourse.mybir as mybir
import concourse.tile as tile
from concourse import bacc
from concourse.bass_utils import run_bass_kernel_spmd

F32 = mybir.dt.float32
F32R = mybir.dt.float32r
AF = mybir.ActivationFunctionType
MUL = mybir.AluOpType.mult

B, L, D = 2, 2048, 4096
H, KH, HD = 32, 8, 128
T = B * L              # 4096 tokens
NC_ = 8                # cores
QH = H // NC_          # 4 q heads per core
EPS = 1e-5
ROPE_BASE = 1000000.0

_CACHE = {}


def _build():
    nc = bacc.Bacc("TRN2", target_bir_lowering=False, debug=False, num_devices=NC_)

    xT = nc.dram_tensor("xT", [D, T], F32R, kind="ExternalInput").ap()
    wq = nc.dram_tensor("wq", [D, QH * HD], F32R, kind="ExternalInput").ap()
    wkv = nc.dram_tensor("wkv", [D, 2 * HD], F32R, kind="ExternalInput").ap()
    wo = nc.dram_tensor("wo", [D, D], F32R, kind="ExternalInput").ap()
    ropeq = nc.dram_tensor("ropeq", [T, 2 * HD], F32, kind="ExternalInput").ap()
    ropek = nc.dram_tensor("ropek", [T, 2 * HD], F32, kind="ExternalInput").ap()
    pat = nc.dram_tensor("pat", [128, 896], F32R, kind="ExternalInput").ap()
    ident = nc.dram_tensor("ident", [128, 128], F32R, kind="ExternalInput").ap()
    ones_c = nc.dram_tensor("ones_c", [128, 1], F32R, kind="ExternalInput").ap()
    ones_r = nc.dram_tensor("ones_r", [1, 128], F32R, kind="ExternalInput").ap()
    out = nc.dram_tensor("out", [T // NC_, D], F32, kind="ExternalOutput").ap()

    xT_r = xT.rearrange("(o p) t -> p o t", p=128)       # [128, 32, T]
    wq_r = wq.rearrange("(o p) n -> p o n", p=128)       # [128, 32, 512]
    wkv_r = wkv.rearrange("(o p) n -> p o n", p=128)     # [128, 32, 256]
    wo_r = wo.rearrange("(o p) n -> p o n", p=128)       # [128, 32, 4096]

    NT = T // 128            # 32 token tiles
    QB = 512                 # q block
    with tile.TileContext(nc) as tc:
        with (
            tc.tile_pool(name="const", bufs=1) as cp,
            tc.tile_pool(name="dram", bufs=1, space="DRAM") as dramp,
            tc.tile_pool(name="kv", bufs=1) as kvp,
        ):
            ident_sb = cp.tile([128, 128], F32R)
            nc.sync.dma_start(ident_sb[:], ident)
            pat_sb = cp.tile([128, 896], F32R)
            nc.sync.dma_start(pat_sb[:], pat)
            onesc_sb = cp.tile([128, 1], F32R)
            nc.sync.dma_start(onesc_sb[:], ones_c)
            onesr_sb = cp.tile([1, 128], F32R)
            nc.sync.dma_start(onesr_sb[:], ones_r)

            kT_sb = kvp.tile([128, T], F32R)          # [hd, tok]
            v_sb = kvp.tile([128, NT, HD], F32R)      # [tok%128, tile, hd]

            qT_d = dramp.tile([QH, 128, T], F32R)
            a2a_in_a = dramp.tile([NC_, 2 * HD, QB], F32R)
            a2a_in_b = dramp.tile([NC_, 2 * HD, QB], F32R)
            a2a_out_a = dramp.tile([NC_, 2 * HD, QB], F32R)
            a2a_out_b = dramp.tile([NC_, 2 * HD, QB], F32R)

            # ---------------- phase 1: projections + norm + rope ----------
            with (
                tc.tile_pool(name="wts", bufs=1) as wp,
                tc.tile_pool(name="p1", bufs=2) as p1,
                tc.tile_pool(name="px", bufs=2) as px,
                tc.tile_pool(name="ps1", bufs=2, space="PSUM") as ps1,
                tc.tile_pool(name="pst", bufs=2, space="PSUM") as pst,
            ):
                wq_sb = wp.tile([128, 32, QH * HD], F32R)
                wkv_sb = wp.tile([128, 32, 2 * HD], F32R)
                for jc in range(4):
                    nc.sync.dma_start(wq_sb[:, 8 * jc:8 * (jc + 1), :],
                                      wq_r[:, 8 * jc:8 * (jc + 1), :])
                    nc.sync.dma_start(wkv_sb[:, 8 * jc:8 * (jc + 1), :],
                                      wkv_r[:, 8 * jc:8 * (jc + 1), :])

                for i in range(NT):
                    xt = px.tile([128, 32, 128], F32R, tag="xt")
                    nc.sync.dma_start(xt[:], xT_r[:, :, 128 * i:128 * (i + 1)])
                    rq = p1.tile([128, 2 * HD], F32, tag="rq")
                    rk = p1.tile([128, 2 * HD], F32, tag="rk")
                    nc.sync.dma_start(rq[:], ropeq[128 * i:128 * (i + 1), :])
                    nc.sync.dma_start(rk[:], ropek[128 * i:128 * (i + 1), :])
                    psq = ps1.tile([128, QH * HD], F32, tag="psq")
                    pskv = ps1.tile([128, 2 * HD], F32, tag="pskv")
                    for j in range(32):
                        xs = xt[:, j, :]
                        nc.tensor.matmul(psq[:], xs, wq_sb[:, j, :],
                                         start=(j == 0), stop=(j == 31))
                        nc.tensor.matmul(pskv[:], xs, wkv_sb[:, j, :],
                                         start=(j == 0), stop=(j == 31))
                    # v: copy to resident (DVE)
                    nc.vector.tensor_copy(out=v_sb[:, i, :], in_=pskv[:, HD:2 * HD])
                    # copy q/k to SBUF, square + reduce on DVE
                    qc = p1.tile([128, QH * HD], F32, tag="qc")
                    nc.vector.tensor_copy(out=qc[:], in_=psq[:])
                    kc = p1.tile([128, HD], F32, tag="kc")
                    nc.vector.tensor_copy(out=kc[:], in_=pskv[:, 0:HD])
                    sq = p1.tile([128, QH * HD], F32, tag="sq")
                    nc.vector.tensor_tensor(sq[:], qc[:], qc[:], MUL)
                    sk = p1.tile([128, HD], F32, tag="sk")
                    nc.vector.tensor_tensor(sk[:], kc[:], kc[:], MUL)
                    ssq = p1.tile([128, QH + 1], F32, tag="ssq")
                    for h in range(QH):
                        nc.vector.reduce_sum(ssq[:, h:h + 1], sq[:, HD * h:HD * (h + 1)],
                                             axis=mybir.AxisListType.X)
                    nc.vector.reduce_sum(ssq[:, QH:QH + 1], sk[:],
                                         axis=mybir.AxisListType.X)
                    var = p1.tile([128, QH + 1], F32, tag="var")
                    nc.vector.tensor_scalar(var[:], ssq[:], 1.0 / HD, EPS,
                                            mybir.AluOpType.mult,
                                            mybir.AluOpType.add)
                    rms = p1.tile([128, QH + 1], F32, tag="rms")
                    nc.scalar.activation(rms[:], var[:], AF.Sqrt)
                    inv = p1.tile([128, QH + 1], F32, tag="inv")
                    nc.vector.reciprocal(inv[:], rms[:])
                    qn = p1.tile([128, QH * HD], F32, tag="qn")
                    for h in range(QH):
                        nc.vector.tensor_scalar_mul(
                            qn[:, HD * h:HD * (h + 1)], qc[:, HD * h:HD * (h + 1)],
                            inv[:, h:h + 1])
                    kn = p1.tile([128, HD], F32, tag="kn")
                    nc.vector.tensor_scalar_mul(kn[:], kc[:], inv[:, QH:QH + 1])

                    # rope; tables carry norm-w (and softmax scale for q)
                    qr = p1.tile([128, QH * HD], F32R, tag="qr")
                    kr = p1.tile([128, HD], F32R, tag="kr")
                    ta = p1.tile([128, 64], F32, tag="ta")
                    tb = p1.tile([128, 64], F32, tag="tb")
                    for (src_t, dst, rt) in [(qn, qr, rq), (kn, kr, rk)]:
                        nh = QH if src_t is qn else 1
                        for h in range(nh):
                            x1 = src_t[:, HD * h:HD * h + 64]
                            x2 = src_t[:, HD * h + 64:HD * h + 128]
                            c1 = rt[:, 0:64]
                            s1 = rt[:, 64:128]
                            c2 = rt[:, 128:192]
                            s2 = rt[:, 192:256]
                            nc.vector.tensor_tensor(ta[:], x1, c1, MUL)
                            nc.vector.tensor_tensor(tb[:], x2, s1, MUL)
                            nc.vector.tensor_tensor(
                                dst[:, HD * h:HD * h + 64], ta[:], tb[:],
                                mybir.AluOpType.subtract)
                            nc.vector.tensor_tensor(ta[:], x2, c2, MUL)
                            nc.vector.tensor_tensor(tb[:], x1, s2, MUL)
                            nc.vector.tensor_tensor(
                                dst[:, HD * h + 64:HD * h + 128], ta[:], tb[:],
                                mybir.AluOpType.add)

                    with nc.allow_low_precision(reason="pure transpose, no accumulation"):
                        qs4 = p1.tile([128, QH, 128], F32R, tag="qs4")
                        for h in range(QH):
                            pt = pst.tile([128, 128], F32R, tag="pt")
                            nc.tensor.transpose(pt[:], qr[:, HD * h:HD * (h + 1)], ident_sb[:])
                            nc.vector.tensor_copy(out=qs4[:, h, :], in_=pt[:])
                        nc.sync.dma_start(
                            qT_d[:, :, 128 * i:128 * (i + 1)].rearrange("h p t -> p h t"),
                            qs4[:])
                        pt = pst.tile([128, 128], F32R, tag="pt")
                        nc.tensor.transpose(pt[:], kr[:], ident_sb[:])
                        nc.vector.tensor_copy(out=kT_sb[:, 128 * i:128 * (i + 1)], in_=pt[:])

            # ---------------- phase 2: attention ---------------------------
            with (
                tc.tile_pool(name="p2", bufs=3) as p2,
                tc.tile_pool(name="ps2", bufs=2, space="PSUM") as ps2,
                tc.tile_pool(name="pso", bufs=2, space="PSUM") as pso,
            ):
                for h in range(QH):
                    for b in range(B):
                        for qb in range(4):
                            q0 = b * L + QB * qb
                            nkt = 4 * qb + 4
                            qt = p2.tile([128, QB], F32R, tag="qt")
                            nc.sync.dma_start(qt[:], qT_d[h, :, q0:q0 + QB])
                            pso_o = pso.tile([128, QB], F32, tag="o")
                            pso_s = pso.tile([1, QB], F32, tag="s")
                            for kt in range(nkt):
                                kc_ = b * L + 128 * kt
                                pss = ps2.tile([128, QB], F32, tag="pss")
                                nc.tensor.matmul(pss[:], kT_sb[:, kc_:kc_ + 128], qt[:],
                                                 start=True, stop=True)
                                pT = p2.tile([128, QB], F32R, tag="pT")
                                nc.scalar.activation(pT[:], pss[:], AF.Exp)
                                t = kt - 4 * qb
                                if t >= 0:
                                    off = 384 - 128 * t
                                    nc.vector.tensor_tensor(
                                        pT[:], pT[:], pat_sb[:, off:off + QB], MUL)
                                nc.tensor.matmul(pso_s[:], onesc_sb[:], pT[:],
                                                 start=(kt == 0), stop=(kt == nkt - 1))
                                nc.tensor.matmul(pso_o[:], v_sb[:, b * 16 + kt, :], pT[:],
                                                 start=(kt == 0), stop=(kt == nkt - 1))
                            rec = p2.tile([1, QB], F32R, tag="rec")
                            with nc.allow_low_precision(reason="f32r tag for bcast matmul"):
                                nc.vector.reciprocal(rec[:], pso_s[:])
                            psb = ps2.tile([128, QB], F32, tag="psb")
                            nc.tensor.matmul(psb[:], onesr_sb[:], rec[:],
                                             start=True, stop=True)
                            bcs = p2.tile([128, QB], F32, tag="bcs")
                            nc.vector.tensor_copy(out=bcs[:], in_=psb[:])
                            attn = p2.tile([128, QB], F32R, tag="attn")
                            nc.vector.tensor_tensor(attn[:], pso_o[:], bcs[:], MUL)
                            j = 4 * b + qb
                            buf = a2a_in_a if h < 2 else a2a_in_b
                            hh = h % 2
                            nc.sync.dma_start(buf[j, HD * hh:HD * (hh + 1), :], attn[:])

            # ---------------- phase 3: all-to-all ---------------------------
            nc.gpsimd.collective_compute(
                "AllToAll", mybir.AluOpType.bypass,
                replica_groups=[list(range(NC_))],
                ins=[a2a_in_a.opt()], outs=[a2a_out_a.opt()])
            nc.gpsimd.collective_compute(
                "AllToAll", mybir.AluOpType.bypass,
                replica_groups=[list(range(NC_))],
                ins=[a2a_in_b.opt()], outs=[a2a_out_b.opt()])

            # ---------------- phase 4: output projection --------------------
            with (
                tc.tile_pool(name="p4", bufs=1) as p4,
                tc.tile_pool(name="p4s", bufs=2) as p4s,
                tc.tile_pool(name="ps4", bufs=2, space="PSUM") as ps4,
            ):
                at_a = p4.tile([128, NC_, 2, QB], F32R)
                at_b = p4.tile([128, NC_, 2, QB], F32R)
                nc.sync.dma_start(at_a[:], a2a_out_a[:].rearrange(
                    "s (d p) t -> p s d t", p=128))
                nc.sync.dma_start(at_b[:], a2a_out_b[:].rearrange(
                    "s (d p) t -> p s d t", p=128))
                for oc in range(8):
                    pso_list = [ps4.tile([128, 512], F32, tag=f"po{tt}", name=f"po{tt}")
                                for tt in range(4)]
                    for kb in range(4):   # kb 0,1 read half A; 2,3 read half B
                        at_half = at_a if kb < 2 else at_b
                        wt = p4s.tile([128, 8, 512], F32R, tag="wt")
                        nc.sync.dma_start(
                            wt[:], wo_r[:, 8 * kb:8 * (kb + 1), 512 * oc:512 * (oc + 1)])
                        for k8 in range(8):
                            k = 8 * kb + k8
                            s, dsub = divmod((kb % 2) * 8 + k8, 2)
                            for tt in range(4):
                                nc.tensor.matmul(
                                    pso_list[tt][:],
                                    at_half[:, s, dsub, 128 * tt:128 * (tt + 1)],
                                    wt[:, k8, :], start=(k == 0), stop=(k == 31))
                    for tt in range(4):
                        ob = p4s.tile([128, 512], F32, tag="ob")
                        nc.vector.tensor_copy(out=ob[:], in_=pso_list[tt][:])
                        nc.sync.dma_start(
                            out[128 * tt:128 * (tt + 1), 512 * oc:512 * (oc + 1)], ob[:])

    nc.compile()
    return nc


def _prep(inputs):
    x = np.asarray(inputs["x"], np.float32)
    wq = np.asarray(inputs["wq"], np.float32)
    wk = np.asarray(inputs["wk"], np.float32)
    wv = np.asarray(inputs["wv"], np.float32)
    wo = np.asarray(inputs["wo"], np.float32)
    qw = np.asarray(inputs["q_norm_w"], np.float32)
    kw = np.asarray(inputs["k_norm_w"], np.float32)

    xT = np.ascontiguousarray(x.reshape(T, D).T)

    half = HD // 2
    inv_freq = 1.0 / (ROPE_BASE ** (np.arange(half, dtype=np.float32) / half))
    pos = np.arange(L, dtype=np.float32)
    ang = pos[:, None] * inv_freq[None, :]
    cos = np.cos(ang).astype(np.float32)
    sin = np.sin(ang).astype(np.float32)
    scale = np.float32(HD ** -0.5)

    def rope_tab(w, s):
        c1 = cos * w[None, 0:half] * s
        s1 = sin * w[None, half:HD] * s
        c2 = cos * w[None, half:HD] * s
        s2 = sin * w[None, 0:half] * s
        t = np.concatenate([c1, s1, c2, s2], axis=1)          # [L, 256]
        return np.ascontiguousarray(np.tile(t, (B, 1)))       # [T, 256]

    ropeq = rope_tab(qw, scale)
    ropek = rope_tab(kw, np.float32(1.0))

    kk = np.arange(128)[:, None]
    cc = np.arange(896)[None, :]
    pat = (kk <= cc - 384).astype(np.float32)
    ident = np.eye(128, dtype=np.float32)
    ones_c = np.ones((128, 1), np.float32)
    ones_r = np.ones((1, 128), np.float32)

    perm = []
    for d4r in (0, 2):
        for s in range(NC_):
            for d4 in (d4r, d4r + 1):
                t0_ = 512 * s + 128 * d4
                perm.extend(range(t0_, t0_ + 128))
    wo_p = np.ascontiguousarray(wo[np.array(perm), :])

    in_maps = []
    for c in range(NC_):
        in_maps.append({
            "xT": xT,
            "wq": np.ascontiguousarray(wq[:, 512 * c:512 * (c + 1)]),
            "wkv": np.ascontiguousarray(
                np.concatenate([wk[:, HD * c:HD * (c + 1)], wv[:, HD * c:HD * (c + 1)]], axis=1)),
            "wo": wo_p,
            "ropeq": ropeq,
            "ropek": ropek,
            "pat": pat,
            "ident": ident,
            "ones_c": ones_c,
            "ones_r": ones_r,
        })
    return in_maps


def kernel(**inputs) -> np.ndarray:
    if "nc" not in _CACHE:
        _CACHE["nc"] = _build()
    nc = _CACHE["nc"]
    in_maps = _prep(inputs)
    res = run_bass_kernel_spmd(nc, in_maps, list(range(NC_)))
    chunks = [res.results[c]["out"] for c in range(NC_)]
    return np.concatenate(chunks, axis=0).reshape(B, L, D)

